# revision 2
# baseline (speedup 1.0000x reference)
"""5G Polar encoder on 8 trn2 cores: 2-column-packed GF(2) matmul.

The whole reference computation is GF(2)-linear in u, so the host composes
one binary matrix M [512, 1024] from the tiny index tables and the device
computes y = (u @ M) mod 2, data-parallel over the batch (8192 rows/core).

v2 packs TWO output columns per PSUM f32:
    y2 = u @ Mlo + u @ (1024*Mhi)
via two accumulating fp8e5m2 DoubleRow matmuls into the same PSUM region.
Everything is exact: operands {0,1} and {0,1024} are e5m2-representable,
products are {0,1,1024}, and sums <= 523 + 523*1024 = 536075 < 2^24 stay
exact in f32 accumulation. This halves PSUM traffic, eviction work, and
output DMA at zero extra TensorE cost (same MAC count, which is the HW
bottleneck: ~54.6us/pass at 157 TF/s fp8-DR).

Eviction per 4-bank PSUM group [128, 2048] (4 batch blocks wide so the
fixed per-op overheads amortize):
  1. ACT Copy f32 -> i32 (bit-exact; f32->i16 would saturate at 32767),
  2. one DVE tensor_scalar AND-1025 on a bitcast i16 stride-2 view of the
     i32 tile (little-endian low halfwords; bitVec ops need matching
     in/out dtypes, the bitcast view satisfies that) -> contiguous i16
     {0,1,1024,1025},
  3. one merged output DMA per group via a rearranged DRAM AP.
Host splits bit0 -> out column j and bit10 -> out column j+512.

HW-verified on silicon: exact (rel err 0.0); per-pass marginal ~54us vs
~65us for the previous i16-unpacked kernel, with ACT ~30us / DVE ~18us /
out-DMA ~23us all comfortably under the TensorE floor.
"""

import numpy as np
import ml_dtypes

N_CORES = 8
BS = 65536
K = 512          # contraction dim (u features)
N = 1024         # final output columns
NPK = N // 2     # 512 packed columns
SHARD = BS // N_CORES  # 8192 batch rows per core
P = 128
KT = K // P      # 4 k-blocks
NB = SHARD // P  # 64 batch blocks per core
GB = 4           # batch blocks per psum group (4 PSUM banks)
NG = NB // GB    # 16 groups per pass
W = GB * NPK     # 2048 free elems per group tile

FP8E5_NP = ml_dtypes.float8_e5m2
LEVEL = 1024

_nc_cache = {}


def build_M(crc_gen, info_pos, ind_gather, perm_out):
    """Compose the encoder into one GF(2) matrix M [K, N]: out = (u @ M) mod 2."""
    crc_gen = np.asarray(crc_gen)
    info_pos = np.asarray(info_pos)
    ind_gather = np.asarray(ind_gather)
    perm_out = np.asarray(perm_out)
    k, _ = crc_gen.shape
    nb, n1 = ind_gather.shape
    kp = info_pos.shape[0]
    C = (crc_gen.astype(np.int64) & 1).astype(np.uint8)
    B = np.concatenate([np.eye(k, dtype=np.uint8), C], axis=1)  # [k, kp]
    # scatter bits into columns; duplicate indices: last write wins (matches
    # jax/numpy .at[].set application order)
    col_src = np.full(n1, -1, np.int64)
    col_src[info_pos] = np.arange(kp)
    A = np.zeros((k, n1), np.uint8)
    valid = col_src >= 0
    A[:, valid] = B[:, col_src[valid]]
    for s in range(nb):
        A = A ^ A[:, ind_gather[s]]
    return A[:, perm_out]  # [k, n]


def _build_nc(reps=1, cvt_act=NG, u_chunks=8, wbufs=3, pbufs=2):
    """cvt_act: how many of the NG groups/pass convert f32->i32 on ACT
    (the rest on DVE; all-ACT is best — DVE already carries the ANDs)."""
    import concourse.tile as tile
    from concourse import bacc, mybir

    nc = bacc.Bacc("TRN2", target_bir_lowering=False, debug=False)
    fp8 = mybir.dt.float8e5
    f32 = mybir.dt.float32
    i32 = mybir.dt.int32
    i16 = mybir.dt.int16
    DR = mybir.MatmulPerfMode.DoubleRow
    A = mybir.AluOpType
    ACTF = mybir.ActivationFunctionType

    # k-major 3D layouts [p, ks, free] with global k = ks*128 + p (both
    # operands use the same mapping, so the contraction is correct).
    uT = nc.declare_dram_parameter("uT", [P, KT, SHARD], fp8, isOutput=False)
    # mat free dim: [0:512] = Mlo {0,1}, [512:1024] = 1024*Mhi {0,1024}
    mat = nc.declare_dram_parameter("mat", [P, KT, N], fp8, isOutput=False)
    y = nc.declare_dram_parameter("y", [SHARD, NPK], i16, isOutput=True)

    with tile.TileContext(nc) as tc:
        with (
            tc.tile_pool(name="consts", bufs=1) as cpool,
            tc.tile_pool(name="work", bufs=wbufs) as wpool,
            tc.tile_pool(name="outs", bufs=wbufs) as opool,
            tc.tile_pool(name="psum", bufs=pbufs, space="PSUM") as ppool,
        ):
            mt = cpool.tile([P, KT, N], fp8, tag="mt")
            nc.sync.dma_start(mt[:], mat[:])
            # chunk the big u load along batch so the first group's matmuls
            # start after ~1/u_chunks of the 4MB has landed
            CW = SHARD // u_chunks
            uts = []
            for c in range(u_chunks):
                ut_c = cpool.tile([P, KT, CW], fp8, tag=f"ut{c}", name=f"ut{c}")
                nc.sync.dma_start(ut_c[:], uT[:, :, c * CW:(c + 1) * CW])
                uts.append(ut_c)
            for i, g in enumerate(
                [g for _ in range(reps) for g in range(NG)]
            ):
                ps = ppool.tile([P, W], f32, tag="ps", name="ps")
                for j in range(GB):
                    b = GB * g + j
                    ut = uts[(b * P) // CW]
                    boff = (b * P) % CW
                    F = slice(j * NPK, (j + 1) * NPK)
                    for ks in range(0, KT, 2):
                        # same stationary u-block serves the lo and hi streams
                        nc.tensor.matmul(
                            ps[:, F],
                            ut[:, ks:ks + 2, boff:boff + P],
                            mt[:, ks:ks + 2, 0:NPK],
                            start=(ks == 0), stop=False, perf_mode=DR,
                        )
                        nc.tensor.matmul(
                            ps[:, F],
                            ut[:, ks:ks + 2, boff:boff + P],
                            mt[:, ks:ks + 2, NPK:N],
                            start=False, stop=(ks == KT - 2), perf_mode=DR,
                        )
                t32 = wpool.tile([P, W], i32, tag="t32")
                ii = i % NG
                if ii * cvt_act // NG != (ii + 1) * cvt_act // NG:
                    nc.scalar.activation(t32[:], ps[:], ACTF.Copy)
                else:
                    nc.vector.tensor_copy(t32[:], ps[:])
                o16 = opool.tile([P, W], i16, tag="o16")
                v16 = t32[:].bitcast(i16)  # [P, 2W] little-endian halfwords
                nc.vector.tensor_scalar(
                    o16[:], v16[:, 0:2 * W:2], (1 | LEVEL), None, A.bitwise_and
                )
                b0 = GB * g
                ydst = y[b0 * P:(b0 + GB) * P, :].rearrange(
                    "(gb p) c -> p gb c", gb=GB, p=P)
                osrc = o16[:].rearrange("p (gb c) -> p gb c", gb=GB, c=NPK)
                nc.sync.dma_start(ydst, osrc)
    nc.compile()
    return nc


CVT_ACT = NG
U_CHUNKS = 8
WBUFS = 3
PBUFS = 2


def get_nc(reps=1):
    key = (reps, CVT_ACT, U_CHUNKS, WBUFS, PBUFS)
    if key not in _nc_cache:
        _nc_cache[key] = _build_nc(reps, cvt_act=CVT_ACT, u_chunks=U_CHUNKS,
                                   wbufs=WBUFS, pbufs=PBUFS)
    return _nc_cache[key]


def _to_k_major(a_km, free):
    """[K, free] -> [P, KT, free] with k = ks*128 + p."""
    return np.ascontiguousarray(
        a_km.reshape(KT, P, free).transpose(1, 0, 2)
    )


def make_in_maps(u, M):
    u8 = np.asarray(u).astype(FP8E5_NP)
    M = np.asarray(M)
    mcomb = np.concatenate(
        [M[:, :NPK], M[:, NPK:].astype(np.float32) * LEVEL], axis=1
    ).astype(FP8E5_NP)
    mat3 = _to_k_major(mcomb, N)
    in_maps = []
    for i in range(N_CORES):
        uT_i = np.ascontiguousarray(u8[i * SHARD:(i + 1) * SHARD, :].T)
        in_maps.append({"uT": _to_k_major(uT_i, SHARD), "mat": mat3})
    return in_maps


def unpack(y2_list):
    """[N_CORES x [SHARD, NPK] i16] -> [BS, N] f32"""
    y2 = np.concatenate([np.asarray(a) for a in y2_list], axis=0)
    lo = (y2 & 1).astype(np.float32)
    hi = ((y2 >> 10) & 1).astype(np.float32)
    return np.concatenate([lo, hi], axis=1)


def kernel(u, crc_gen, info_pos, ind_gather, perm_out):
    from concourse.bass_utils import run_bass_kernel_spmd

    M = build_M(crc_gen, info_pos, ind_gather, perm_out)
    in_maps = make_in_maps(u, M)
    nc = get_nc()
    res = run_bass_kernel_spmd(nc, in_maps, core_ids=list(range(N_CORES)))
    return unpack([r["y"] for r in res.results])
